# revision 2
# baseline (speedup 1.0000x reference)
"""Trainium2 Bass kernel for DCKModule (involution) — v3.

Channel-partition layout (like the baseline), re-engineered around the
TimelineSim cost model:

  x   = relu(W1' @ guide + bias)            PE bf16 (1cyc/row) + Act
  dk  = W2exp_k @ x                         PE bf16 (4x faster than the
        (256 rows, gc-broadcast built        baseline's fp32 4cyc/row),
         into W2exp row replication)         pairs of taps per 3-bank
                                             PSUM tile
  per tap (di,dj) of the 7x7 kernel:
    prod = dk * f_shift                     dj0-3: Act converts dk pair ->
                                             bf16 SBUF, DVE mults at 2x
                                             (2x_1p mode), dj-batched in
                                             one fat instruction
                                            dj4: converted, mult on Pool
                                            dj5,6: DVE mult fp32 from
                                             PSUM directly (1x)
    acc += prod                             mostly PE identity-matmul
                                             accumulate into PSUM (1cyc/
                                             row); dj6 chain on Pool
                                             (seeded with the residual);
                                             some dj5 on a DVE bf16 chain
  out = acc_psum + pool_acc (+ dve_acc)     DVE drains, DMA out

Data-parallel: 1 image per core, 8 cores. f/guide/weights shipped bf16
from the host; output fp32.  Processing in 12 chunks of 8 image rows x 2
channel-halves (768 px, PSUM acc = 2 banks; dk pair tiles 2x3 banks).
"""

import numpy as np
import ml_dtypes

import concourse.bass as bass
import concourse.mybir as mybir
import concourse.tile as tile
from concourse import bacc, bass_utils

B, C, H, W = 8, 256, 96, 96
K7, PAD, G, GC, R = 7, 3, 16, 16, 64
HP = H + 2 * PAD          # 102
PIX = H * W               # 9216
PPIX = HP * HP            # 10404
BN_EPS = 1e-5
RB = 8                    # image rows per chunk
NCH = H // RB             # 12 chunks
CHW = RB * W              # 768 px per chunk
XB = 512                  # phase-1 pixel block

F32 = mybir.dt.float32
BF16 = mybir.dt.bfloat16
AluOp = mybir.AluOpType
Act = mybir.ActivationFunctionType
TRACE = False

# per-dj engine assignment for the multiply:
#   'B' = DVE bf16 (needs Act convert), 'P' = Pool bf16 (needs convert),
#   'F' = DVE fp32 straight from PSUM
MULT_ENG = ['B', 'B', 'B', 'B', 'P', 'F', 'F']
N_BF = 5                  # dkb slots (converted taps dj 0..4)
# add engine per (di, dj): dj4 -> Pool chain (self-contained with its
# Pool mult); dj5 -> DVE chain on DVE_ADD_DI rows
POOL_ADD_DJ = 4
DVE_ADD_DJ = 5
DVE_ADD_DI = (0, 1, 2, 3, 4, 5, 6)    # which di rows put dj5's add on the DVE chain

_CACHE = {}


def _build_nc():
    nc = bacc.Bacc(None, target_bir_lowering=False)
    fm_d = nc.dram_tensor("fm", [C, PPIX], BF16, kind="ExternalInput")
    gm_d = nc.dram_tensor("gm", [C, PIX], BF16, kind="ExternalInput")
    w1_d = nc.dram_tensor("w1t", [C, R], BF16, kind="ExternalInput")
    bias_d = nc.dram_tensor("bias", [R, 1], F32, kind="ExternalInput")
    w2_d = nc.dram_tensor("w2e", [R, 49 * C], BF16, kind="ExternalInput")
    id_d = nc.dram_tensor("ident", [128, 128], BF16, kind="ExternalInput")
    out_d = nc.dram_tensor("out", [C, PIX], F32, kind="ExternalOutput")

    with tile.TileContext(nc) as tc:
        with tc.tile_pool(name="persist", bufs=1) as persist, \
             tc.tile_pool(name="gpool", bufs=2) as gpool, \
             tc.tile_pool(name="dkbpool", bufs=2) as dkbpool, \
             tc.tile_pool(name="prodpool", bufs=3) as prodpool, \
             tc.tile_pool(name="chainpool", bufs=2) as chainpool, \
             tc.tile_pool(name="outpool", bufs=2) as outpool:

            f_sb = [persist.tile([128, PPIX], BF16, tag=f"f{ct}",
                                 name=f"fsb{ct}") for ct in range(2)]
            w1_sb = persist.tile([128, 2 * R], BF16, tag="w1", name="w1sb")
            bias_sb = persist.tile([R, 1], F32, tag="bias", name="biassb")
            w2_sb = persist.tile([R, 49 * C], BF16, tag="w2", name="w2sb")
            id_sb = persist.tile([128, 128], BF16, tag="id", name="idsb")
            x_sb = persist.tile([R, PIX], BF16, tag="x", name="xsb")

            for ct in range(2):
                nc.gpsimd.dma_start(out=f_sb[ct][:],
                                    in_=fm_d[ct * 128:(ct + 1) * 128, :])
            for ck in range(2):
                nc.gpsimd.dma_start(out=w1_sb[:, ck * R:(ck + 1) * R],
                                    in_=w1_d[ck * 128:(ck + 1) * 128, :])
            nc.gpsimd.dma_start(out=bias_sb[:], in_=bias_d[:])
            nc.gpsimd.dma_start(out=w2_sb[:], in_=w2_d[:])
            nc.gpsimd.dma_start(out=id_sb[:], in_=id_d[:])

            with tc.tile_pool(name="psx", bufs=2, space="PSUM") as psx:
                # observer matmuls: PE consumes the gpsimd DMA-queue sems
                # one at a time so no later Matmult needs 2 queue waits
                obs = psx.tile([1, 2], F32, tag="obs", name="obs", bufs=1)
                nc.tensor.matmul(obs[:, 0:1], w1_sb[:, 0:1], w1_sb[:, 0:1],
                                 start=True, stop=True)
                nc.tensor.matmul(obs[:, 0:1], w2_sb[:, 0:1], w2_sb[:, 0:1],
                                 start=True, stop=True)
                nc.tensor.matmul(obs[:, 1:2], id_sb[:, 0:1], id_sb[:, 0:1],
                                 start=True, stop=True)
                vobs = persist.tile([128, 3], F32, tag="vobs", name="vobs")
                nc.vector.tensor_copy(vobs[:R, 0:1], bias_sb[:])
                nc.vector.tensor_copy(vobs[:, 1:2], f_sb[0][:, 0:1])
                nc.vector.tensor_copy(vobs[:, 2:3], f_sb[1][:, 0:1])

                # phase 1: x = relu(W1' @ guide + bias) -> bf16 [64, 9216]
                for blk in range(PIX // XB):
                    g_sb = [gpool.tile([128, XB], BF16, tag=f"g{ct}",
                                       name=f"gsb{ct}") for ct in range(2)]
                    for ct in range(2):
                        nc.sync.dma_start(
                            out=g_sb[ct][:],
                            in_=gm_d[ct * 128:(ct + 1) * 128,
                                     blk * XB:(blk + 1) * XB])
                    px = psx.tile([R, XB], F32, tag="px", name="px")
                    for ck in range(2):
                        nc.tensor.matmul(
                            px[:], w1_sb[:, ck * R:(ck + 1) * R],
                            g_sb[ck][:], start=(ck == 0), stop=(ck == 1))
                    nc.scalar.activation(
                        x_sb[:, blk * XB:(blk + 1) * XB], px[:],
                        Act.Relu, bias=bias_sb[:], scale=1.0)

            with tc.tile_pool(name="psdk", bufs=3, space="PSUM") as psdk, \
                 tc.tile_pool(name="psacc", bufs=1, space="PSUM") as psacc:

                def add_eng(di, dj):
                    if dj == POOL_ADD_DJ:
                        return 'P'
                    if dj == DVE_ADD_DJ and di in DVE_ADD_DI:
                        return 'D'
                    return 'PE'

                last_pe = [(di, dj) for di in range(K7) for dj in range(K7)
                           if add_eng(di, dj) == 'PE'][-1]

                for ch in range(NCH):
                    r0 = ch * RB
                    xs = x_sb[:, r0 * W:(r0 + RB) * W]
                    for ct in range(2):
                        acc = psacc.tile([128, CHW], F32, tag="acc",
                                         name="acc")
                        pacc = chainpool.tile([128, CHW], BF16, tag="pacc",
                                              name="pacc")
                        dacc = chainpool.tile([128, CHW], BF16, tag="dacc",
                                              name="dacc")
                        pe_state = {"first": True}
                        dve_first = True

                        def fslice(di, dj, r0=r0, ct=ct, nd=1):
                            """f operand view: nd dj-taps batched"""
                            dims = [[PPIX, 128]]
                            if nd > 1:
                                dims.append([1, nd])
                            dims += [[HP, RB], [1, W]]
                            return bass.AP(
                                f_sb[ct].tensor,
                                f_sb[ct][:].offset + (r0 + di) * HP + dj,
                                dims)

                        def pe_add(pt, off, stop):
                            for lo, hi in ((0, 512), (512, 768)):
                                nc.tensor.matmul(
                                    acc[:, lo:hi], id_sb[:],
                                    pt[:, off + lo:off + hi],
                                    start=pe_state["first"], stop=stop,
                                    skip_group_check=True)
                            pe_state["first"] = False

                        for di in range(K7):
                            dkb = dkbpool.tile([128, N_BF * CHW], BF16,
                                               tag="dkb", name="dkb")
                            dks = {}

                            def produce(dj, di=di, ct=ct, dks=dks,
                                        dkb=dkb, xs=xs):
                                t = psdk.tile([128, CHW], F32, tag="dkp",
                                              name="dkp")
                                dks[dj] = t
                                kk = di * K7 + dj
                                wsl = w2_sb[:, kk * C + ct * 128:
                                            kk * C + ct * 128 + 128]
                                for lo, hi in ((0, 512), (512, 768)):
                                    nc.tensor.matmul(
                                        t[:, lo:hi], wsl, xs[:, lo:hi],
                                        start=True, stop=True,
                                        skip_group_check=True)
                                if MULT_ENG[dj] in ('B', 'P'):
                                    nc.scalar.activation(
                                        dkb[:, dj * CHW:(dj + 1) * CHW],
                                        t[:], Act.Copy, scale=1.0)

                            def bmult(dj0, nd, di=di, ct=ct, dkb=dkb):
                                pt = prodpool.tile(
                                    [128, nd * CHW], BF16,
                                    tag=f"pb{dj0}", name=f"pb{dj0}")
                                if nd > 1:
                                    ov = pt[:].rearrange(
                                        "p (d r l) -> p d r l", d=nd, l=W)
                                    sv = dkb[:, dj0 * CHW:
                                             (dj0 + nd) * CHW].rearrange(
                                        "p (d r l) -> p d r l", d=nd, l=W)
                                else:
                                    ov = pt[:].rearrange(
                                        "p (r l) -> p r l", l=W)
                                    sv = dkb[:, dj0 * CHW:(dj0 + 1) * CHW
                                             ].rearrange(
                                        "p (r l) -> p r l", l=W)
                                eng = (nc.gpsimd if MULT_ENG[dj0] == 'P'
                                       else nc.vector)
                                eng.tensor_tensor(
                                    ov, sv, fslice(di, dj0, nd=nd),
                                    AluOp.mult)
                                return pt

                            def fmult(dj, di=di, ct=ct, dks=dks):
                                pt = prodpool.tile([128, CHW], BF16,
                                                   tag=f"ps{dj}",
                                                   name=f"ps{dj}")
                                nc.vector.tensor_tensor(
                                    pt[:].rearrange("p (r l) -> p r l",
                                                    l=W),
                                    dks[dj][:].rearrange(
                                        "p (r l) -> p r l", l=W),
                                    fslice(di, dj), AluOp.mult)
                                return pt

                            # interleaved emission: produce/convert pairs,
                            # multiply as soon as inputs land, add after
                            produce(5)
                            p5 = fmult(5)
                            produce(0)
                            produce(1)
                            pb01 = bmult(0, 2)
                            produce(6)
                            p6 = fmult(6)
                            pe_add(pb01, 0, False)
                            pe_add(pb01, CHW, False)
                            produce(4)
                            pb4 = bmult(4, 1)          # Pool mult
                            produce(2)
                            produce(3)
                            pb23 = bmult(2, 2)
                            pe_add(pb23, 0, False)
                            pe_add(pb23, CHW, False)

                            # Pool chain: dj4 (self-contained on Pool)
                            if di == 0:
                                rsd = bass.AP(
                                    f_sb[ct].tensor,
                                    f_sb[ct][:].offset
                                    + (r0 + PAD) * HP + PAD,
                                    [[PPIX, 128], [HP, RB], [1, W]])
                                nc.gpsimd.tensor_tensor(
                                    pacc[:].rearrange("p (r l) -> p r l",
                                                      l=W),
                                    pb4[:].rearrange("p (r l) -> p r l",
                                                     l=W),
                                    rsd, AluOp.add)
                            else:
                                nc.gpsimd.tensor_tensor(
                                    pacc[:], pacc[:], pb4[:], AluOp.add)

                            # dj5: DVE chain on some rows, else PE
                            if add_eng(di, DVE_ADD_DJ) == 'D':
                                if dve_first:
                                    nc.vector.tensor_copy(dacc[:], p5[:])
                                    dve_first = False
                                else:
                                    nc.vector.tensor_tensor(
                                        dacc[:], dacc[:], p5[:],
                                        AluOp.add)
                            else:
                                pe_add(p5, 0, (di, 5) == last_pe)
                            pe_add(p6, 0, (di, 6) == last_pe)

                        # --- drain: out = acc + pacc + dacc
                        osb = outpool.tile([128, CHW], F32, tag="osb",
                                           name="osb")
                        nc.vector.tensor_tensor(osb[:], acc[:], pacc[:],
                                                AluOp.add)
                        nc.vector.tensor_tensor(osb[:], osb[:], dacc[:],
                                                AluOp.add)
                        nc.sync.dma_start(
                            out=out_d[ct * 128:(ct + 1) * 128,
                                      r0 * W:(r0 + RB) * W],
                            in_=osb[:])
    if not nc.is_finalized():
        nc.finalize()
    return nc


def _host_prep(feature_map, guide_map, W1, bn_gamma, bn_beta, bn_mean,
               bn_var, W2):
    fm = np.asarray(feature_map, np.float32)
    gm = np.asarray(guide_map, np.float32)
    inv = bn_gamma / np.sqrt(bn_var + BN_EPS)
    w1t = np.ascontiguousarray((W1 * inv[:, None]).T).astype(
        ml_dtypes.bfloat16)                                  # [256, 64]
    bias = (bn_beta - bn_mean * inv).astype(np.float32).reshape(R, 1)
    W2r = np.asarray(W2, np.float32).reshape(G, 49, R)       # [g, k, o]
    w2e = np.ascontiguousarray(
        np.repeat(W2r.transpose(2, 1, 0)[:, :, :, None], GC, axis=3)
        .reshape(R, 49 * C)).astype(ml_dtypes.bfloat16)      # [o, k*256+c]
    fpad = np.pad(fm, ((0, 0), (0, 0), (PAD, PAD), (PAD, PAD))).reshape(
        B, C, PPIX).astype(ml_dtypes.bfloat16)
    gmb = gm.reshape(B, C, PIX).astype(ml_dtypes.bfloat16)
    ident = np.eye(128, dtype=ml_dtypes.bfloat16)
    return fpad, gmb, w1t, bias, w2e, ident


def kernel(feature_map, guide_map, W1, bn_gamma, bn_beta, bn_mean, bn_var,
           W2):
    fpad, gmb, w1t, bias, w2e, ident = _host_prep(
        feature_map, guide_map, np.asarray(W1, np.float32),
        np.asarray(bn_gamma, np.float32), np.asarray(bn_beta, np.float32),
        np.asarray(bn_mean, np.float32), np.asarray(bn_var, np.float32),
        np.asarray(W2, np.float32))

    if "nc" not in _CACHE:
        _CACHE["nc"] = _build_nc()
    nc = _CACHE["nc"]

    in_maps = [dict(fm=fpad[i], gm=gmb[i], w1t=w1t, bias=bias, w2e=w2e,
                    ident=ident) for i in range(B)]
    _CACHE["in_maps"] = in_maps
    res = bass_utils.run_bass_kernel_spmd(
        nc, in_maps, core_ids=list(range(B)), trace=TRACE)
    _CACHE["last"] = res
    out = np.stack([np.asarray(r["out"], np.float32)
                    for r in res.results], axis=0)
    return out.reshape(B, C, H, W)


# revision 3
# speedup vs baseline: 1.0193x; 1.0193x over previous
"""Trainium2 Bass kernel for DCKModule (involution) — v3.

Channel-partition layout (like the baseline), re-engineered around the
TimelineSim cost model:

  x   = relu(W1' @ guide + bias)            PE bf16 (1cyc/row) + Act
  dk  = W2exp_k @ x                         PE bf16 (4x faster than the
        (256 rows, gc-broadcast built        baseline's fp32 4cyc/row),
         into W2exp row replication)         pairs of taps per 3-bank
                                             PSUM tile
  per tap (di,dj) of the 7x7 kernel:
    prod = dk * f_shift                     dj0-3: Act converts dk pair ->
                                             bf16 SBUF, DVE mults at 2x
                                             (2x_1p mode), dj-batched in
                                             one fat instruction
                                            dj4: converted, mult on Pool
                                            dj5,6: DVE mult fp32 from
                                             PSUM directly (1x)
    acc += prod                             mostly PE identity-matmul
                                             accumulate into PSUM (1cyc/
                                             row); dj6 chain on Pool
                                             (seeded with the residual);
                                             some dj5 on a DVE bf16 chain
  out = acc_psum + pool_acc (+ dve_acc)     DVE drains, DMA out

Data-parallel: 1 image per core, 8 cores. f/guide/weights shipped bf16
from the host; output fp32.  Processing in 12 chunks of 8 image rows x 2
channel-halves (768 px, PSUM acc = 2 banks; dk pair tiles 2x3 banks).
"""

import numpy as np
import ml_dtypes

import concourse.bass as bass
import concourse.mybir as mybir
import concourse.tile as tile
from concourse import bacc, bass_utils

B, C, H, W = 8, 256, 96, 96
K7, PAD, G, GC, R = 7, 3, 16, 16, 64
HP = H + 2 * PAD          # 102
PIX = H * W               # 9216
PPIX = HP * HP            # 10404
BN_EPS = 1e-5
RB = 8                    # image rows per chunk
NCH = H // RB             # 12 chunks
CHW = RB * W              # 768 px per chunk
XB = 512                  # phase-1 pixel block

F32 = mybir.dt.float32
BF16 = mybir.dt.bfloat16
AluOp = mybir.AluOpType
Act = mybir.ActivationFunctionType
TRACE = False

# per-dj engine assignment for the multiply:
#   'B' = DVE bf16 (needs Act convert), 'P' = Pool bf16 (needs convert),
#   'F' = DVE fp32 straight from PSUM
MULT_ENG = ['B', 'B', 'B', 'B', 'P', 'F', 'F']
N_BF = 5                  # dkb slots (converted taps dj 0..4)
# add engine per (di, dj): dj4 -> Pool chain (self-contained with its
# Pool mult); dj5 -> DVE chain on DVE_ADD_DI rows
POOL_ADD_DJ = 4
DVE_ADD_DJ = 5
DVE_ADD_DI = (0, 1, 2, 3, 4, 5, 6)    # which di rows put dj5's add on the DVE chain

_CACHE = {}


def _build_nc():
    nc = bacc.Bacc(None, target_bir_lowering=False)
    fm_d = nc.dram_tensor("fm", [C, PPIX], BF16, kind="ExternalInput")
    gm_d = nc.dram_tensor("gm", [C, PIX], BF16, kind="ExternalInput")
    w1_d = nc.dram_tensor("w1t", [C, R], BF16, kind="ExternalInput")
    bias_d = nc.dram_tensor("bias", [R, 1], F32, kind="ExternalInput")
    w2_d = nc.dram_tensor("w2e", [R, 49 * C], BF16, kind="ExternalInput")
    id_d = nc.dram_tensor("ident", [128, 128], BF16, kind="ExternalInput")
    out_d = nc.dram_tensor("out", [C, PIX], F32, kind="ExternalOutput")

    with tile.TileContext(nc) as tc:
        with tc.tile_pool(name="persist", bufs=1) as persist, \
             tc.tile_pool(name="gpool", bufs=3) as gpool, \
             tc.tile_pool(name="dkbpool", bufs=2) as dkbpool, \
             tc.tile_pool(name="prodpool", bufs=3) as prodpool, \
             tc.tile_pool(name="chainpool", bufs=2) as chainpool, \
             tc.tile_pool(name="outpool", bufs=2) as outpool:

            f_sb = [persist.tile([128, PPIX], BF16, tag=f"f{ct}",
                                 name=f"fsb{ct}") for ct in range(2)]
            w1_sb = persist.tile([128, 2 * R], BF16, tag="w1", name="w1sb")
            bias_sb = persist.tile([R, 1], F32, tag="bias", name="biassb")
            w2_sb = persist.tile([R, 49 * C], BF16, tag="w2", name="w2sb")
            id_sb = persist.tile([128, 128], BF16, tag="id", name="idsb")
            x_sb = persist.tile([R, PIX], BF16, tag="x", name="xsb")

            for ct in range(2):
                nc.gpsimd.dma_start(out=f_sb[ct][:],
                                    in_=fm_d[ct * 128:(ct + 1) * 128, :])
            for ck in range(2):
                nc.gpsimd.dma_start(out=w1_sb[:, ck * R:(ck + 1) * R],
                                    in_=w1_d[ck * 128:(ck + 1) * 128, :])
            nc.gpsimd.dma_start(out=bias_sb[:], in_=bias_d[:])
            nc.gpsimd.dma_start(out=w2_sb[:], in_=w2_d[:])
            nc.gpsimd.dma_start(out=id_sb[:], in_=id_d[:])

            with tc.tile_pool(name="psx", bufs=2, space="PSUM") as psx:
                # observer matmuls: PE consumes the gpsimd DMA-queue sems
                # one at a time so no later Matmult needs 2 queue waits
                obs = psx.tile([1, 2], F32, tag="obs", name="obs", bufs=1)
                nc.tensor.matmul(obs[:, 0:1], w1_sb[:, 0:1], w1_sb[:, 0:1],
                                 start=True, stop=True)
                nc.tensor.matmul(obs[:, 0:1], w2_sb[:, 0:1], w2_sb[:, 0:1],
                                 start=True, stop=True)
                nc.tensor.matmul(obs[:, 1:2], id_sb[:, 0:1], id_sb[:, 0:1],
                                 start=True, stop=True)
                vobs = persist.tile([128, 3], F32, tag="vobs", name="vobs")
                nc.vector.tensor_copy(vobs[:R, 0:1], bias_sb[:])
                nc.vector.tensor_copy(vobs[:, 1:2], f_sb[0][:, 0:1])
                nc.vector.tensor_copy(vobs[:, 2:3], f_sb[1][:, 0:1])

                # phase 1: x = relu(W1' @ guide + bias) -> bf16 [64, 9216]
                for blk in range(PIX // XB):
                    g_sb = [gpool.tile([128, XB], BF16, tag=f"g{ct}",
                                       name=f"gsb{ct}") for ct in range(2)]
                    for ct in range(2):
                        nc.sync.dma_start(
                            out=g_sb[ct][:],
                            in_=gm_d[ct * 128:(ct + 1) * 128,
                                     blk * XB:(blk + 1) * XB])
                    px = psx.tile([R, XB], F32, tag="px", name="px")
                    for ck in range(2):
                        nc.tensor.matmul(
                            px[:], w1_sb[:, ck * R:(ck + 1) * R],
                            g_sb[ck][:], start=(ck == 0), stop=(ck == 1))
                    nc.scalar.activation(
                        x_sb[:, blk * XB:(blk + 1) * XB], px[:],
                        Act.Relu, bias=bias_sb[:], scale=1.0)

            with tc.tile_pool(name="psdk", bufs=3, space="PSUM") as psdk, \
                 tc.tile_pool(name="psacc", bufs=1, space="PSUM") as psacc:

                def add_eng(di, dj):
                    if dj == POOL_ADD_DJ:
                        return 'P'
                    if dj == DVE_ADD_DJ and di in DVE_ADD_DI:
                        return 'D'
                    return 'PE'

                last_pe = [(di, dj) for di in range(K7) for dj in range(K7)
                           if add_eng(di, dj) == 'PE'][-1]

                for ch in range(NCH):
                    r0 = ch * RB
                    xs = x_sb[:, r0 * W:(r0 + RB) * W]
                    for ct in range(2):
                        acc = psacc.tile([128, CHW], F32, tag="acc",
                                         name="acc")
                        pacc = chainpool.tile([128, CHW], BF16, tag="pacc",
                                              name="pacc")
                        dacc = chainpool.tile([128, CHW], BF16, tag="dacc",
                                              name="dacc")
                        pe_state = {"first": True}
                        dve_first = True

                        def fslice(di, dj, r0=r0, ct=ct, nd=1):
                            """f operand view: nd dj-taps batched"""
                            dims = [[PPIX, 128]]
                            if nd > 1:
                                dims.append([1, nd])
                            dims += [[HP, RB], [1, W]]
                            return bass.AP(
                                f_sb[ct].tensor,
                                f_sb[ct][:].offset + (r0 + di) * HP + dj,
                                dims)

                        def pe_add(pt, off, stop):
                            for lo, hi in ((0, 512), (512, 768)):
                                nc.tensor.matmul(
                                    acc[:, lo:hi], id_sb[:],
                                    pt[:, off + lo:off + hi],
                                    start=pe_state["first"], stop=stop,
                                    skip_group_check=True)
                            pe_state["first"] = False

                        for di in range(K7):
                            dkb = dkbpool.tile([128, N_BF * CHW], BF16,
                                               tag="dkb", name="dkb")
                            dks = {}

                            def produce(dj, di=di, ct=ct, dks=dks,
                                        dkb=dkb, xs=xs):
                                t = psdk.tile([128, CHW], F32, tag="dkp",
                                              name="dkp")
                                dks[dj] = t
                                kk = di * K7 + dj
                                wsl = w2_sb[:, kk * C + ct * 128:
                                            kk * C + ct * 128 + 128]
                                for lo, hi in ((0, 512), (512, 768)):
                                    nc.tensor.matmul(
                                        t[:, lo:hi], wsl, xs[:, lo:hi],
                                        start=True, stop=True,
                                        skip_group_check=True)
                                if MULT_ENG[dj] in ('B', 'P'):
                                    nc.scalar.activation(
                                        dkb[:, dj * CHW:(dj + 1) * CHW],
                                        t[:], Act.Copy, scale=1.0)

                            def bmult(dj0, nd, di=di, ct=ct, dkb=dkb):
                                pt = prodpool.tile(
                                    [128, nd * CHW], BF16,
                                    tag=f"pb{dj0}", name=f"pb{dj0}")
                                if nd > 1:
                                    ov = pt[:].rearrange(
                                        "p (d r l) -> p d r l", d=nd, l=W)
                                    sv = dkb[:, dj0 * CHW:
                                             (dj0 + nd) * CHW].rearrange(
                                        "p (d r l) -> p d r l", d=nd, l=W)
                                else:
                                    ov = pt[:].rearrange(
                                        "p (r l) -> p r l", l=W)
                                    sv = dkb[:, dj0 * CHW:(dj0 + 1) * CHW
                                             ].rearrange(
                                        "p (r l) -> p r l", l=W)
                                eng = (nc.gpsimd if MULT_ENG[dj0] == 'P'
                                       else nc.vector)
                                eng.tensor_tensor(
                                    ov, sv, fslice(di, dj0, nd=nd),
                                    AluOp.mult)
                                return pt

                            def fmult(dj, di=di, ct=ct, dks=dks):
                                pt = prodpool.tile([128, CHW], BF16,
                                                   tag=f"ps{dj}",
                                                   name=f"ps{dj}")
                                nc.vector.tensor_tensor(
                                    pt[:].rearrange("p (r l) -> p r l",
                                                    l=W),
                                    dks[dj][:].rearrange(
                                        "p (r l) -> p r l", l=W),
                                    fslice(di, dj), AluOp.mult)
                                return pt

                            # interleaved emission: produce/convert pairs,
                            # multiply as soon as inputs land, add after
                            produce(5)
                            p5 = fmult(5)
                            produce(0)
                            produce(1)
                            pb01 = bmult(0, 2)
                            produce(6)
                            p6 = fmult(6)
                            pe_add(pb01, 0, False)
                            pe_add(pb01, CHW, False)
                            produce(4)
                            pb4 = bmult(4, 1)          # Pool mult
                            produce(2)
                            produce(3)
                            pb23 = bmult(2, 2)
                            pe_add(pb23, 0, False)
                            pe_add(pb23, CHW, False)

                            # Pool chain: dj4 (self-contained on Pool)
                            if di == 0:
                                rsd = bass.AP(
                                    f_sb[ct].tensor,
                                    f_sb[ct][:].offset
                                    + (r0 + PAD) * HP + PAD,
                                    [[PPIX, 128], [HP, RB], [1, W]])
                                nc.gpsimd.tensor_tensor(
                                    pacc[:].rearrange("p (r l) -> p r l",
                                                      l=W),
                                    pb4[:].rearrange("p (r l) -> p r l",
                                                     l=W),
                                    rsd, AluOp.add)
                            else:
                                nc.gpsimd.tensor_tensor(
                                    pacc[:], pacc[:], pb4[:], AluOp.add)

                            # dj5: DVE chain on some rows, else PE
                            if add_eng(di, DVE_ADD_DJ) == 'D':
                                if dve_first:
                                    nc.vector.tensor_copy(dacc[:], p5[:])
                                    dve_first = False
                                else:
                                    nc.vector.tensor_tensor(
                                        dacc[:], dacc[:], p5[:],
                                        AluOp.add)
                            else:
                                pe_add(p5, 0, (di, 5) == last_pe)
                            pe_add(p6, 0, (di, 6) == last_pe)

                        # --- drain: out = acc + pacc + dacc
                        osb = outpool.tile([128, CHW], F32, tag="osb",
                                           name="osb")
                        nc.vector.tensor_tensor(osb[:], acc[:], pacc[:],
                                                AluOp.add)
                        nc.gpsimd.tensor_tensor(osb[:], osb[:], dacc[:],
                                                AluOp.add)
                        nc.sync.dma_start(
                            out=out_d[ct * 128:(ct + 1) * 128,
                                      r0 * W:(r0 + RB) * W],
                            in_=osb[:])
    if not nc.is_finalized():
        nc.finalize()
    return nc


def _host_prep(feature_map, guide_map, W1, bn_gamma, bn_beta, bn_mean,
               bn_var, W2):
    fm = np.asarray(feature_map, np.float32)
    gm = np.asarray(guide_map, np.float32)
    inv = bn_gamma / np.sqrt(bn_var + BN_EPS)
    w1t = np.ascontiguousarray((W1 * inv[:, None]).T).astype(
        ml_dtypes.bfloat16)                                  # [256, 64]
    bias = (bn_beta - bn_mean * inv).astype(np.float32).reshape(R, 1)
    W2r = np.asarray(W2, np.float32).reshape(G, 49, R)       # [g, k, o]
    w2e = np.ascontiguousarray(
        np.repeat(W2r.transpose(2, 1, 0)[:, :, :, None], GC, axis=3)
        .reshape(R, 49 * C)).astype(ml_dtypes.bfloat16)      # [o, k*256+c]
    fpad = np.pad(fm, ((0, 0), (0, 0), (PAD, PAD), (PAD, PAD))).reshape(
        B, C, PPIX).astype(ml_dtypes.bfloat16)
    gmb = gm.reshape(B, C, PIX).astype(ml_dtypes.bfloat16)
    ident = np.eye(128, dtype=ml_dtypes.bfloat16)
    return fpad, gmb, w1t, bias, w2e, ident


def kernel(feature_map, guide_map, W1, bn_gamma, bn_beta, bn_mean, bn_var,
           W2):
    fpad, gmb, w1t, bias, w2e, ident = _host_prep(
        feature_map, guide_map, np.asarray(W1, np.float32),
        np.asarray(bn_gamma, np.float32), np.asarray(bn_beta, np.float32),
        np.asarray(bn_mean, np.float32), np.asarray(bn_var, np.float32),
        np.asarray(W2, np.float32))

    if "nc" not in _CACHE:
        _CACHE["nc"] = _build_nc()
    nc = _CACHE["nc"]

    in_maps = [dict(fm=fpad[i], gm=gmb[i], w1t=w1t, bias=bias, w2e=w2e,
                    ident=ident) for i in range(B)]
    _CACHE["in_maps"] = in_maps
    res = bass_utils.run_bass_kernel_spmd(
        nc, in_maps, core_ids=list(range(B)), trace=TRACE)
    _CACHE["last"] = res
    out = np.stack([np.asarray(r["out"], np.float32)
                    for r in res.results], axis=0)
    return out.reshape(B, C, H, W)


# revision 4
# speedup vs baseline: 1.0289x; 1.0094x over previous
"""Trainium2 Bass kernel for DCKModule (involution) — v3.

Channel-partition layout (like the baseline), re-engineered around the
TimelineSim cost model:

  x   = relu(W1' @ guide + bias)            PE bf16 (1cyc/row) + Act
  dk  = W2exp_k @ x                         PE bf16 (4x faster than the
        (256 rows, gc-broadcast built        baseline's fp32 4cyc/row),
         into W2exp row replication)         pairs of taps per 3-bank
                                             PSUM tile
  per tap (di,dj) of the 7x7 kernel:
    prod = dk * f_shift                     dj0-3: Act converts dk pair ->
                                             bf16 SBUF, DVE mults at 2x
                                             (2x_1p mode), dj-batched in
                                             one fat instruction
                                            dj4: converted, mult on Pool
                                            dj5,6: DVE mult fp32 from
                                             PSUM directly (1x)
    acc += prod                             mostly PE identity-matmul
                                             accumulate into PSUM (1cyc/
                                             row); dj6 chain on Pool
                                             (seeded with the residual);
                                             some dj5 on a DVE bf16 chain
  out = acc_psum + pool_acc (+ dve_acc)     DVE drains, DMA out

Data-parallel: 1 image per core, 8 cores. f/guide/weights shipped bf16
from the host; output fp32.  Processing in 12 chunks of 8 image rows x 2
channel-halves (768 px, PSUM acc = 2 banks; dk pair tiles 2x3 banks).
"""

import numpy as np
import ml_dtypes

import concourse.bass as bass
import concourse.mybir as mybir
import concourse.tile as tile
from concourse import bacc, bass_utils

B, C, H, W = 8, 256, 96, 96
K7, PAD, G, GC, R = 7, 3, 16, 16, 64
HP = H + 2 * PAD          # 102
PIX = H * W               # 9216
PPIX = HP * HP            # 10404
BN_EPS = 1e-5
RB = 8                    # image rows per chunk
NCH = H // RB             # 12 chunks
CHW = RB * W              # 768 px per chunk
XB = 1024                  # phase-1 pixel block

F32 = mybir.dt.float32
BF16 = mybir.dt.bfloat16
AluOp = mybir.AluOpType
Act = mybir.ActivationFunctionType
TRACE = False

# per-dj engine assignment for the multiply:
#   'B' = DVE bf16 (needs Act convert), 'P' = Pool bf16 (needs convert),
#   'F' = DVE fp32 straight from PSUM
MULT_ENG = ['B', 'B', 'B', 'B', 'P', 'F', 'F']
N_BF = 5                  # dkb slots (converted taps dj 0..4)
# add engine per (di, dj): dj4 -> Pool chain (self-contained with its
# Pool mult); dj5 -> DVE chain on DVE_ADD_DI rows
POOL_ADD_DJ = 4
DVE_ADD_DJ = 5
DVE_ADD_DI = (0, 1, 2, 3, 4, 5, 6)    # which di rows put dj5's add on the DVE chain

_CACHE = {}


def _build_nc():
    nc = bacc.Bacc(None, target_bir_lowering=False)
    fm_d = nc.dram_tensor("fm", [C, PPIX], BF16, kind="ExternalInput")
    gm_d = nc.dram_tensor("gm", [C, PIX], BF16, kind="ExternalInput")
    w1_d = nc.dram_tensor("w1t", [C, R], BF16, kind="ExternalInput")
    bias_d = nc.dram_tensor("bias", [R, 1], F32, kind="ExternalInput")
    w2_d = nc.dram_tensor("w2e", [R, 49 * C], BF16, kind="ExternalInput")
    id_d = nc.dram_tensor("ident", [128, 128], BF16, kind="ExternalInput")
    out_d = nc.dram_tensor("out", [C, PIX], F32, kind="ExternalOutput")

    with tile.TileContext(nc) as tc:
        with tc.tile_pool(name="persist", bufs=1) as persist, \
             tc.tile_pool(name="gpool", bufs=3) as gpool, \
             tc.tile_pool(name="dkbpool", bufs=2) as dkbpool, \
             tc.tile_pool(name="prodpool", bufs=3) as prodpool, \
             tc.tile_pool(name="chainpool", bufs=2) as chainpool, \
             tc.tile_pool(name="outpool", bufs=2) as outpool:

            f_sb = [persist.tile([128, PPIX], BF16, tag=f"f{ct}",
                                 name=f"fsb{ct}") for ct in range(2)]
            w1_sb = persist.tile([128, 2 * R], BF16, tag="w1", name="w1sb")
            bias_sb = persist.tile([R, 1], F32, tag="bias", name="biassb")
            w2_sb = persist.tile([R, 49 * C], BF16, tag="w2", name="w2sb")
            id_sb = persist.tile([128, 128], BF16, tag="id", name="idsb")
            x_sb = persist.tile([R, PIX], BF16, tag="x", name="xsb")

            for ct in range(2):
                nc.gpsimd.dma_start(out=f_sb[ct][:],
                                    in_=fm_d[ct * 128:(ct + 1) * 128, :])
            for ck in range(2):
                nc.gpsimd.dma_start(out=w1_sb[:, ck * R:(ck + 1) * R],
                                    in_=w1_d[ck * 128:(ck + 1) * 128, :])
            nc.gpsimd.dma_start(out=bias_sb[:], in_=bias_d[:])
            nc.gpsimd.dma_start(out=w2_sb[:], in_=w2_d[:])
            nc.gpsimd.dma_start(out=id_sb[:], in_=id_d[:])

            with tc.tile_pool(name="psx", bufs=3, space="PSUM") as psx:
                # observer matmuls: PE consumes the gpsimd DMA-queue sems
                # one at a time so no later Matmult needs 2 queue waits
                obs = psx.tile([1, 2], F32, tag="obs", name="obs", bufs=1)
                nc.tensor.matmul(obs[:, 0:1], w1_sb[:, 0:1], w1_sb[:, 0:1],
                                 start=True, stop=True)
                nc.tensor.matmul(obs[:, 0:1], w2_sb[:, 0:1], w2_sb[:, 0:1],
                                 start=True, stop=True)
                nc.tensor.matmul(obs[:, 1:2], id_sb[:, 0:1], id_sb[:, 0:1],
                                 start=True, stop=True)
                vobs = persist.tile([128, 3], F32, tag="vobs", name="vobs")
                nc.vector.tensor_copy(vobs[:R, 0:1], bias_sb[:])
                nc.vector.tensor_copy(vobs[:, 1:2], f_sb[0][:, 0:1])
                nc.vector.tensor_copy(vobs[:, 2:3], f_sb[1][:, 0:1])

                # phase 1: x = relu(W1' @ guide + bias) -> bf16 [64, 9216]
                for blk in range(PIX // XB):
                    g_sb = [gpool.tile([128, XB], BF16, tag=f"g{ct}",
                                       name=f"gsb{ct}") for ct in range(2)]
                    for ct in range(2):
                        nc.sync.dma_start(
                            out=g_sb[ct][:],
                            in_=gm_d[ct * 128:(ct + 1) * 128,
                                     blk * XB:(blk + 1) * XB])
                    px = psx.tile([R, XB], F32, tag="px", name="px")
                    for s in range(XB // 512):
                        for ck in range(2):
                            nc.tensor.matmul(
                                px[:, s * 512:(s + 1) * 512],
                                w1_sb[:, ck * R:(ck + 1) * R],
                                g_sb[ck][:, s * 512:(s + 1) * 512],
                                start=(ck == 0), stop=(ck == 1))
                    nc.scalar.activation(
                        x_sb[:, blk * XB:(blk + 1) * XB], px[:],
                        Act.Relu, bias=bias_sb[:], scale=1.0)

            with tc.tile_pool(name="psdk", bufs=3, space="PSUM") as psdk, \
                 tc.tile_pool(name="psacc", bufs=1, space="PSUM") as psacc:

                def add_eng(di, dj):
                    if dj == POOL_ADD_DJ:
                        return 'P'
                    if dj == DVE_ADD_DJ and di in DVE_ADD_DI:
                        return 'D'
                    return 'PE'

                last_pe = [(di, dj) for di in range(K7) for dj in range(K7)
                           if add_eng(di, dj) == 'PE'][-1]

                for ch in range(NCH):
                    r0 = ch * RB
                    xs = x_sb[:, r0 * W:(r0 + RB) * W]
                    for ct in range(2):
                        acc = psacc.tile([128, CHW], F32, tag="acc",
                                         name="acc")
                        pacc = chainpool.tile([128, CHW], BF16, tag="pacc",
                                              name="pacc")
                        dacc = chainpool.tile([128, CHW], BF16, tag="dacc",
                                              name="dacc")
                        pe_state = {"first": True}
                        dve_first = True

                        def fslice(di, dj, r0=r0, ct=ct, nd=1):
                            """f operand view: nd dj-taps batched"""
                            dims = [[PPIX, 128]]
                            if nd > 1:
                                dims.append([1, nd])
                            dims += [[HP, RB], [1, W]]
                            return bass.AP(
                                f_sb[ct].tensor,
                                f_sb[ct][:].offset + (r0 + di) * HP + dj,
                                dims)

                        def pe_add(pt, off, stop):
                            for lo, hi in ((0, 512), (512, 768)):
                                nc.tensor.matmul(
                                    acc[:, lo:hi], id_sb[:],
                                    pt[:, off + lo:off + hi],
                                    start=pe_state["first"], stop=stop,
                                    skip_group_check=True)
                            pe_state["first"] = False

                        for di in range(K7):
                            dkb = dkbpool.tile([128, N_BF * CHW], BF16,
                                               tag="dkb", name="dkb")
                            dks = {}

                            def produce(dj, di=di, ct=ct, dks=dks,
                                        dkb=dkb, xs=xs):
                                t = psdk.tile([128, CHW], F32, tag="dkp",
                                              name="dkp")
                                dks[dj] = t
                                kk = di * K7 + dj
                                wsl = w2_sb[:, kk * C + ct * 128:
                                            kk * C + ct * 128 + 128]
                                for lo, hi in ((0, 512), (512, 768)):
                                    nc.tensor.matmul(
                                        t[:, lo:hi], wsl, xs[:, lo:hi],
                                        start=True, stop=True,
                                        skip_group_check=True)
                                if MULT_ENG[dj] in ('B', 'P'):
                                    nc.scalar.activation(
                                        dkb[:, dj * CHW:(dj + 1) * CHW],
                                        t[:], Act.Copy, scale=1.0)

                            def bmult(dj0, nd, di=di, ct=ct, dkb=dkb):
                                pt = prodpool.tile(
                                    [128, nd * CHW], BF16,
                                    tag=f"pb{dj0}", name=f"pb{dj0}")
                                if nd > 1:
                                    ov = pt[:].rearrange(
                                        "p (d r l) -> p d r l", d=nd, l=W)
                                    sv = dkb[:, dj0 * CHW:
                                             (dj0 + nd) * CHW].rearrange(
                                        "p (d r l) -> p d r l", d=nd, l=W)
                                else:
                                    ov = pt[:].rearrange(
                                        "p (r l) -> p r l", l=W)
                                    sv = dkb[:, dj0 * CHW:(dj0 + 1) * CHW
                                             ].rearrange(
                                        "p (r l) -> p r l", l=W)
                                eng = (nc.gpsimd if MULT_ENG[dj0] == 'P'
                                       else nc.vector)
                                eng.tensor_tensor(
                                    ov, sv, fslice(di, dj0, nd=nd),
                                    AluOp.mult)
                                return pt

                            def fmult(dj, di=di, ct=ct, dks=dks):
                                pt = prodpool.tile([128, CHW], BF16,
                                                   tag=f"ps{dj}",
                                                   name=f"ps{dj}")
                                nc.vector.tensor_tensor(
                                    pt[:].rearrange("p (r l) -> p r l",
                                                    l=W),
                                    dks[dj][:].rearrange(
                                        "p (r l) -> p r l", l=W),
                                    fslice(di, dj), AluOp.mult)
                                return pt

                            # interleaved emission: produce/convert pairs,
                            # multiply as soon as inputs land, add after
                            produce(5)
                            p5 = fmult(5)
                            produce(0)
                            produce(1)
                            pb01 = bmult(0, 2)
                            produce(6)
                            p6 = fmult(6)
                            pe_add(pb01, 0, False)
                            pe_add(pb01, CHW, False)
                            produce(4)
                            pb4 = bmult(4, 1)          # Pool mult
                            produce(2)
                            produce(3)
                            pb23 = bmult(2, 2)
                            pe_add(pb23, 0, False)
                            pe_add(pb23, CHW, False)

                            # Pool chain: dj4 (self-contained on Pool)
                            if di == 0:
                                rsd = bass.AP(
                                    f_sb[ct].tensor,
                                    f_sb[ct][:].offset
                                    + (r0 + PAD) * HP + PAD,
                                    [[PPIX, 128], [HP, RB], [1, W]])
                                nc.gpsimd.tensor_tensor(
                                    pacc[:].rearrange("p (r l) -> p r l",
                                                      l=W),
                                    pb4[:].rearrange("p (r l) -> p r l",
                                                     l=W),
                                    rsd, AluOp.add)
                            else:
                                nc.gpsimd.tensor_tensor(
                                    pacc[:], pacc[:], pb4[:], AluOp.add)

                            # dj5: DVE chain on some rows, else PE
                            if add_eng(di, DVE_ADD_DJ) == 'D':
                                if dve_first:
                                    nc.vector.tensor_copy(dacc[:], p5[:])
                                    dve_first = False
                                else:
                                    nc.vector.tensor_tensor(
                                        dacc[:], dacc[:], p5[:],
                                        AluOp.add)
                            else:
                                pe_add(p5, 0, (di, 5) == last_pe)
                            pe_add(p6, 0, (di, 6) == last_pe)

                        # --- drain: out = acc + pacc + dacc
                        osb = outpool.tile([128, CHW], F32, tag="osb",
                                           name="osb")
                        nc.vector.tensor_tensor(osb[:], acc[:], pacc[:],
                                                AluOp.add)
                        nc.gpsimd.tensor_tensor(osb[:], osb[:], dacc[:],
                                                AluOp.add)
                        nc.sync.dma_start(
                            out=out_d[ct * 128:(ct + 1) * 128,
                                      r0 * W:(r0 + RB) * W],
                            in_=osb[:])
    if not nc.is_finalized():
        nc.finalize()
    return nc


def _host_prep(feature_map, guide_map, W1, bn_gamma, bn_beta, bn_mean,
               bn_var, W2):
    fm = np.asarray(feature_map, np.float32)
    gm = np.asarray(guide_map, np.float32)
    inv = bn_gamma / np.sqrt(bn_var + BN_EPS)
    w1t = np.ascontiguousarray((W1 * inv[:, None]).T).astype(
        ml_dtypes.bfloat16)                                  # [256, 64]
    bias = (bn_beta - bn_mean * inv).astype(np.float32).reshape(R, 1)
    W2r = np.asarray(W2, np.float32).reshape(G, 49, R)       # [g, k, o]
    w2e = np.ascontiguousarray(
        np.repeat(W2r.transpose(2, 1, 0)[:, :, :, None], GC, axis=3)
        .reshape(R, 49 * C)).astype(ml_dtypes.bfloat16)      # [o, k*256+c]
    fpad = np.pad(fm, ((0, 0), (0, 0), (PAD, PAD), (PAD, PAD))).reshape(
        B, C, PPIX).astype(ml_dtypes.bfloat16)
    gmb = gm.reshape(B, C, PIX).astype(ml_dtypes.bfloat16)
    ident = np.eye(128, dtype=ml_dtypes.bfloat16)
    return fpad, gmb, w1t, bias, w2e, ident


def kernel(feature_map, guide_map, W1, bn_gamma, bn_beta, bn_mean, bn_var,
           W2):
    fpad, gmb, w1t, bias, w2e, ident = _host_prep(
        feature_map, guide_map, np.asarray(W1, np.float32),
        np.asarray(bn_gamma, np.float32), np.asarray(bn_beta, np.float32),
        np.asarray(bn_mean, np.float32), np.asarray(bn_var, np.float32),
        np.asarray(W2, np.float32))

    if "nc" not in _CACHE:
        _CACHE["nc"] = _build_nc()
    nc = _CACHE["nc"]

    in_maps = [dict(fm=fpad[i], gm=gmb[i], w1t=w1t, bias=bias, w2e=w2e,
                    ident=ident) for i in range(B)]
    _CACHE["in_maps"] = in_maps
    res = bass_utils.run_bass_kernel_spmd(
        nc, in_maps, core_ids=list(range(B)), trace=TRACE)
    _CACHE["last"] = res
    out = np.stack([np.asarray(r["out"], np.float32)
                    for r in res.results], axis=0)
    return out.reshape(B, C, H, W)
